# revision 15
# baseline (speedup 1.0000x reference)
"""DGCNN (4 EdgeConv layers + projection + global max) Trainium2 Bass kernel.

Data-parallel over batch: 16 samples -> 8 NeuronCores x 2 samples.

Math per EdgeConv layer (exactly equivalent to the reference):
  y[n,j,:] = W_n x_j + (W_c - W_n) x_n  for j in kNN(n)   (1x1 conv on [x_j-x_n; x_n])
  out[n,:] = lrelu(s * (max_j U[j,:] + V[n,:]) + b),  U = X W_n^T, V = X (W_c-W_n)^T
(s = g*BN_INV_STD >= 0, so the max over neighbors commutes with the monotone
 affine + LeakyReLU.)

kNN selection: D_sel[n,m] = 2<x_n,x_m> - |x_m|^2 (row-constant -|x_n|^2 dropped
-- same top-k). Top-40 per row via 5 rounds of DVE max8/max_index/match_replace.
Neighbor aggregation: indirect DMA gather of U rows + max-tree on GPSIMD.
"""

import numpy as np

B, N, KNN = 16, 1024, 40
HD = [64, 64, 128, 256]
INC = [3, 64, 64, 128]
NCORES, SPC = 8, 2
BN_INV = float(1.0 / np.sqrt(1.0 + 1e-5))
NEG = -1e30
SH = sum(HD)  # 512
Z2 = 1024

_CACHE = {}


def _build_program():
    from contextlib import ExitStack

    import concourse.bacc as bacc
    import concourse.bass as bass
    import concourse.tile as tile
    from concourse import mybir
    from concourse.bass import IndirectOffsetOnAxis, _add_dep_helper

    f32 = mybir.dt.float32
    u16 = mybir.dt.uint16
    i16 = mybir.dt.int16
    Alu = mybir.AluOpType
    Act = mybir.ActivationFunctionType
    Ax = mybir.AxisListType

    nc = bacc.Bacc("TRN2", target_bir_lowering=False, debug=False,
                   enable_asserts=False, num_devices=NCORES)

    # ---- DRAM I/O ----
    xt_d = nc.dram_tensor("xt", [SPC, 3, N], f32, kind="ExternalInput")
    wn_d, wv_d, s_d, b_d = [], [], [], []
    for li in range(4):
        C, Co = INC[li], HD[li]
        wn_d.append(nc.dram_tensor(f"wn{li}", [C, Co], f32, kind="ExternalInput"))
        wv_d.append(nc.dram_tensor(f"wv{li}", [C, Co], f32, kind="ExternalInput"))
        nch = (Co + 127) // 128
        s_d.append(nc.dram_tensor(f"s{li}", [min(Co, 128), nch], f32, kind="ExternalInput"))
        b_d.append(nc.dram_tensor(f"b{li}", [min(Co, 128), nch], f32, kind="ExternalInput"))
    # wf split per layer-chunk, transposed: [C_l, 1024]
    WFT_SPEC = [64, 64, 128, 128, 128]
    wft_d = [nc.dram_tensor(f"wft{i}", [ck, Z2], f32, kind="ExternalInput")
             for i, ck in enumerate(WFT_SPEC)]
    bf_d = nc.dram_tensor("bf2d", [128, 8], f32, kind="ExternalInput")
    eye_d = nc.dram_tensor("eye", [128, 128], f32, kind="ExternalInput")
    out_d = nc.dram_tensor("out", [SPC, 128, 8], f32, kind="ExternalOutput")
    u_hbm = [[nc.dram_tensor(f"u_{s}_{li}", [N, HD[li]], f32, kind="Internal")
              for li in range(4)] for s in range(SPC)]
    # per-tile gather index lists in the SWDGE 16-wrap layout [16, 320]
    ih_hbm = [[nc.dram_tensor(f"ih_{s}_{li}", [N // 128, 16, 320], u16,
                              kind="Internal") for li in range(4)]
              for s in range(SPC)]

    NT = N // 128  # 8 point-tiles

    with tile.TileContext(nc) as tc, ExitStack() as ctx:
        pool = ctx.enter_context(tc.tile_pool(name="main", bufs=1))
        pD = ctx.enter_context(tc.tile_pool(name="pD", bufs=3))
        pX = ctx.enter_context(tc.tile_pool(name="pX", bufs=2))
        pG = ctx.enter_context(tc.tile_pool(name="pG", bufs=2))
        pV = ctx.enter_context(tc.tile_pool(name="pV", bufs=2))
        pU = ctx.enter_context(tc.tile_pool(name="pU", bufs=2))
        pSm = ctx.enter_context(tc.tile_pool(name="pSm", bufs=2))
        pNs = ctx.enter_context(tc.tile_pool(name="pNs", bufs=1))
        pPd = ctx.enter_context(tc.tile_pool(name="pPd", bufs=1, space="PSUM"))
        pPm = ctx.enter_context(tc.tile_pool(name="pPm", bufs=2, space="PSUM"))
        pPb = ctx.enter_context(tc.tile_pool(name="pPb", bufs=1, space="PSUM"))

        # ---- persistent constants ----
        xt_sb = []
        for s in range(SPC):
            t = pool.tile([3, N], f32, tag=f"xt{s}")
            nc.sync.dma_start(t[:], xt_d[s])
            xt_sb.append(t)
        wn_sb, wv_sb, s_sb, b_sb = [], [], [], []
        for li in range(4):
            C, Co = INC[li], HD[li]
            t = pool.tile([C, Co], f32, tag=f"wn{li}")
            nc.sync.dma_start(t[:], wn_d[li][:])
            wn_sb.append(t)
            t = pool.tile([C, Co], f32, tag=f"wv{li}")
            nc.sync.dma_start(t[:], wv_d[li][:])
            wv_sb.append(t)
            nch = (Co + 127) // 128
            t = pool.tile([min(Co, 128), nch], f32, tag=f"s{li}")
            nc.sync.dma_start(t[:], s_d[li][:])
            s_sb.append(t)
            t = pool.tile([min(Co, 128), nch], f32, tag=f"b{li}")
            nc.sync.dma_start(t[:], b_d[li][:])
            b_sb.append(t)
        wft_sb = []
        for i, ck in enumerate(WFT_SPEC):
            t = pool.tile([ck, Z2], f32, tag=f"wft{i}")
            nc.sync.dma_start(t[:], wft_d[i][:])
            wft_sb.append(t)
        bf_sb = pool.tile([128, 8], f32, tag="bf")
        nc.sync.dma_start(bf_sb[:], bf_d[:])
        eye = pool.tile([128, 128], f32, tag="eye")
        nc.sync.dma_start(eye[:], eye_d[:])
        ones1 = pool.tile([1, 128], f32, tag="ones1")
        nc.vector.memset(ones1[:], 1.0)
        negones = pool.tile([128, 1], f32, tag="negones")
        nc.vector.memset(negones[:], -1.0)
        twos = pool.tile([128, 1], f32, tag="twos")
        nc.vector.memset(twos[:], 2.0)

        # layer outputs (transposed [C,1024]), persistent: per sample,
        # l1,l2 -> [64,1024]; l3 -> [128,1024]; l4 -> 2x[128,1024]
        cat_sb = [[pool.tile([64, N], f32, tag=f"o0_{s}", name=f"o0_{s}"),
                   pool.tile([64, N], f32, tag=f"o1_{s}", name=f"o1_{s}"),
                   pool.tile([128, N], f32, tag=f"o2_{s}", name=f"o2_{s}"),
                   pool.tile([128, N], f32, tag=f"o3a_{s}", name=f"o3a_{s}"),
                   pool.tile([128, N], f32, tag=f"o3b_{s}", name=f"o3b_{s}")]
                  for s in range(SPC)]

        def layer(s, li, XT):
            """XT: [C, 1024] sbuf AP for this layer's input (transposed)."""
            C, Co = INC[li], HD[li]
            nch = (Co + 127) // 128
            # ---- prep: Xsq, negsq, X2 ----
            xsq = pD.tile([C, N], f32, tag="d")
            nc.scalar.activation(xsq[:], XT, Act.Square)
            sq_ps = pPb.tile([1, N], f32, tag="big")
            for h in range(2):
                nc.tensor.matmul(sq_ps[:, 512 * h:512 * (h + 1)], negones[0:C, :],
                                 xsq[:, 512 * h:512 * (h + 1)], start=True, stop=True)
            nsq = pNs.tile([1, N], f32, tag="nsq")
            nc.scalar.activation(nsq[:], sq_ps[:], Act.Identity)
            x2 = pX.tile([C, N], f32, tag="x2")
            nc.scalar.activation(x2[:], XT, Act.Identity, scale=twos[0:C, :])

            # ---- U = X Wn^T  (point-major [128,Co] tiles) -> HBM ----
            uw_insts = []
            for t in range(NT):
                ups = pPb.tile([128, Co], f32, tag="big")
                nc.tensor.matmul(ups[:], XT[:, 128 * t:128 * (t + 1)], wn_sb[li][:],
                                 start=True, stop=True)
                usb = pU.tile([128, Co], f32, tag="u")
                nc.scalar.activation(usb[:], ups[:], Act.Identity)
                uw = nc.sync.dma_start(u_hbm[s][li][128 * t:128 * (t + 1), :], usb[:])
                uw_insts.append(uw)

            # ---- V' = s*V + b (transposed [Co,1024]) ----
            vp = []
            for a in range(nch):
                ca = min(128, Co - 128 * a)
                vps = pPb.tile([128, N], f32, tag="big")
                for h in range(2):
                    nc.tensor.matmul(vps[0:ca, 512 * h:512 * (h + 1)],
                                     wv_sb[li][:, 128 * a:128 * a + ca],
                                     XT[:, 512 * h:512 * (h + 1)],
                                     start=True, stop=True)
                vsb = pV.tile([ca, N], f32, tag="v")
                nc.scalar.activation(vsb[:], vps[0:ca, :], Act.Identity,
                                     scale=s_sb[li][0:ca, a:a + 1],
                                     bias=b_sb[li][0:ca, a:a + 1])
                vp.append(vsb)

            # ---- per point-tile: D, top-40, gather, neighbor max ----
            mps = [pPm.tile([128, N], f32, tag="m", name=f"m_{s}_{li}_{a}")
                   for a in range(nch)]
            for t in range(NT):
                dps = pPd.tile([128, N], f32, tag="dps")
                for h in range(2):
                    hs = slice(512 * h, 512 * (h + 1))
                    nc.tensor.matmul(dps[:, hs], x2[:, 128 * t:128 * (t + 1)],
                                     XT[:, hs], start=True, stop=False)
                    nc.tensor.matmul(dps[:, hs], ones1[:], nsq[:, hs],
                                     start=False, stop=True)
                dsb = pD.tile([128, N], f32, tag="d")
                nc.scalar.activation(dsb[:], dps[:], Act.Identity)
                idx = pSm.tile([128, 128], u16, tag="idx")
                nc.vector.memset(idx[:, KNN:], 0)
                for q in range(5):
                    mv = pSm.tile([128, 8], f32, tag="mv")
                    nc.vector.max(out=mv[:], in_=dsb[:])
                    nc.vector.max_index(out=idx[:, 8 * q:8 * q + 8], in_max=mv[:],
                                        in_values=dsb[:])
                    if q < 4:
                        nc.vector.match_replace(out=dsb[:], in_to_replace=mv[:],
                                                in_values=dsb[:], imm_value=NEG)
                # rewrap indices [128 pts, 40] -> SWDGE 16-wrap [16, 320] via
                # XBAR transpose + free-dim permute + HBM bounce
                idxT = pSm.tile([128, 128], u16, tag="idxT")
                nc.sync.dma_start_transpose(idxT[:], idx[:])
                idxP = pSm.tile([KNN, 128], u16, tag="idxP")
                nc.vector.tensor_copy(
                    idxP[:].rearrange("m (s c8) -> m s c8", c8=8),
                    idxT[0:KNN, :].rearrange("m (c8 s) -> m s c8", s=16))
                ih_v = ih_hbm[s][li][t].rearrange("s (m c8) -> m s c8", c8=8)
                ihw = nc.sync.dma_start(ih_v, idxP[:].rearrange(
                    "m (s c8) -> m s c8", c8=8))
                idxs_sb = pSm.tile([128, 320], i16, tag="idxsb")
                for g in range(8):
                    rd = nc.sync.dma_start(idxs_sb[16 * g:16 * (g + 1), :],
                                           ih_hbm[s][li][t].bitcast(i16))
                    _add_dep_helper(rd.ins, ihw.ins, sync=True, reason="ih RAW")
                gb = pG.tile([128, KNN * Co], f32, tag="gb")
                g3 = gb[:].rearrange("p (r c) -> p r c", r=KNN)
                for q in range(5):  # 1024 descriptors per SWDGE call (ring cap)
                    gi = nc.gpsimd.dma_gather(
                        out_ap=g3[:, 8 * q:8 * (q + 1), :], in_ap=u_hbm[s][li][:],
                        idxs_ap=idxs_sb[:, 64 * q:64 * (q + 1)],
                        num_idxs=1024, num_idxs_reg=1024, elem_size=Co)
                    for uw in uw_insts:
                        _add_dep_helper(gi.ins, uw.ins, sync=True, reason="u_hbm RAW")
                # max over 40 neighbor rows (DVE reduce over strided view)
                mrow = pU.tile([128, Co], f32, tag="mrow")
                nc.vector.tensor_reduce(
                    out=mrow[:], in_=gb[:].rearrange("p (r c) -> p c r", r=KNN),
                    axis=Ax.X, op=Alu.max)
                # transpose M rows [128, Co] -> M^T columns
                for a in range(nch):
                    ca = min(128, Co - 128 * a)
                    nc.tensor.transpose(mps[a][0:ca, 128 * t:128 * (t + 1)],
                                        mrow[:, 128 * a:128 * a + ca], eye[:])

            # ---- epilogue: out = lrelu(s*M^T + V') ----
            outs = []
            for a in range(nch):
                ca = min(128, Co - 128 * a)
                o = _otile(s, li, a)
                nc.vector.scalar_tensor_tensor(out=o[:], in0=mps[a][0:ca, :],
                                               scalar=s_sb[li][0:ca, a:a + 1],
                                               in1=vp[a][:], op0=Alu.mult, op1=Alu.add)
                nc.vector.scalar_tensor_tensor(out=o[:], in0=o[:], scalar=0.2,
                                               in1=o[:], op0=Alu.mult, op1=Alu.max)
                outs.append(o)
            return outs

        def _otile(s, li, a):
            return cat_sb[s][li + a if li < 3 else 3 + a]

        for li in range(4):
            for s in range(SPC):
                if li == 0:
                    layer(s, 0, xt_sb[s][:])
                elif li == 3:
                    layer(s, 3, cat_sb[s][2][:])
                else:
                    layer(s, li, cat_sb[s][li - 1][:])

        # ---- final: y = cat @ wf^T + bf ; out = max over points ----
        for s in range(SPC):
            ycat = pSm.tile([128, 8], f32, tag="ycat")
            rhs = [cat_sb[s][0], cat_sb[s][1], cat_sb[s][2], cat_sb[s][3], cat_sb[s][4]]
            for o in range(8):
                yps = pPb.tile([128, N], f32, tag="big")
                for h in range(2):
                    hs = slice(512 * h, 512 * (h + 1))
                    for k in range(5):
                        nc.tensor.matmul(yps[:, hs],
                                         wft_sb[k][:, 128 * o:128 * (o + 1)],
                                         rhs[k][:, hs], start=(k == 0),
                                         stop=(k == 4))
                nc.vector.tensor_reduce(out=ycat[:, o:o + 1], in_=yps[:],
                                        axis=Ax.X, op=Alu.max)
            nc.vector.tensor_tensor(out=ycat[:], in0=ycat[:], in1=bf_sb[:],
                                    op=Alu.add)
            nc.sync.dma_start(out_d[s], ycat[:])

    nc.compile()
    return nc


def _host_inputs(inputs):
    """Build the per-core input maps from the full problem inputs."""
    x = np.asarray(inputs["x"], dtype=np.float32)
    maps = []
    base = {}
    for li in range(4):
        C, Co = INC[li], HD[li]
        w = np.asarray(inputs[f"w{li + 1}"], dtype=np.float32)
        g = np.asarray(inputs[f"g{li + 1}"], dtype=np.float32)
        b = np.asarray(inputs[f"b{li + 1}"], dtype=np.float32)
        wn = w[:, :C]
        wv = w[:, C:] - wn
        base[f"wn{li}"] = np.ascontiguousarray(wn.T)
        base[f"wv{li}"] = np.ascontiguousarray(wv.T)
        nch = (Co + 127) // 128
        base[f"s{li}"] = np.ascontiguousarray(
            (g * BN_INV).astype(np.float32).reshape(nch, -1).T)
        base[f"b{li}"] = np.ascontiguousarray(b.reshape(nch, -1).T)
    wf = np.asarray(inputs["wf"], dtype=np.float32)
    offs = np.cumsum([0] + HD)
    pieces = [wf[:, offs[i]:offs[i + 1]] for i in range(4)]
    wfts = [pieces[0], pieces[1], pieces[2], pieces[3][:, :128], pieces[3][:, 128:]]
    for i, p in enumerate(wfts):
        base[f"wft{i}"] = np.ascontiguousarray(p.T)
    bf = np.asarray(inputs["bf"], dtype=np.float32)
    base["bf2d"] = np.ascontiguousarray(bf.reshape(8, 128).T)
    base["eye"] = np.eye(128, dtype=np.float32)
    for c in range(NCORES):
        m = dict(base)
        xs = x[SPC * c:SPC * (c + 1)]  # [SPC, N, 3]
        m["xt"] = np.ascontiguousarray(np.transpose(xs, (0, 2, 1)))
        maps.append(m)
    return maps


def kernel(**inputs):
    from concourse.bass_utils import run_bass_kernel_spmd

    if "nc" not in _CACHE:
        _CACHE["nc"] = _build_program()
    nc = _CACHE["nc"]
    in_maps = _host_inputs(inputs)
    res = run_bass_kernel_spmd(nc, in_maps, list(range(NCORES)))
    out = np.empty((B, Z2), dtype=np.float32)
    for c in range(NCORES):
        y = res.results[c]["out"]  # [SPC, 128, 8]
        for s in range(SPC):
            out[SPC * c + s] = y[s].T.reshape(Z2)
    return out


# revision 17
# speedup vs baseline: 1.0006x; 1.0006x over previous
"""DGCNN (4 EdgeConv layers + projection + global max) Trainium2 Bass kernel.

Data-parallel over batch: 16 samples -> 8 NeuronCores x 2 samples.

Math per EdgeConv layer (exactly equivalent to the reference):
  y[n,j,:] = W_n x_j + (W_c - W_n) x_n  for j in kNN(n)   (1x1 conv on [x_j-x_n; x_n])
  out[n,:] = lrelu(s * (max_j U[j,:] + V[n,:]) + b),  U = X W_n^T, V = X (W_c-W_n)^T
(s = g*BN_INV_STD >= 0, so the max over neighbors commutes with the monotone
 affine + LeakyReLU.)

kNN selection: D_sel[n,m] = 2<x_n,x_m> - |x_m|^2 (row-constant -|x_n|^2 dropped
-- same top-k). Top-40 per row via 5 rounds of DVE max8/max_index/match_replace.
Neighbor aggregation: indirect DMA gather of U rows + max-tree on GPSIMD.
"""

import numpy as np

B, N, KNN = 16, 1024, 40
HD = [64, 64, 128, 256]
INC = [3, 64, 64, 128]
NCORES, SPC = 8, 2
BN_INV = float(1.0 / np.sqrt(1.0 + 1e-5))
NEG = -1e30
SH = sum(HD)  # 512
Z2 = 1024

_CACHE = {}


def _build_program():
    from contextlib import ExitStack

    import concourse.bacc as bacc
    import concourse.tile as tile
    from concourse import mybir
    from concourse.bass import _add_dep_helper

    f32 = mybir.dt.float32
    u16 = mybir.dt.uint16
    i16 = mybir.dt.int16
    Alu = mybir.AluOpType
    Act = mybir.ActivationFunctionType
    Ax = mybir.AxisListType

    nc = bacc.Bacc("TRN2", target_bir_lowering=False, debug=False,
                   enable_asserts=False, num_devices=NCORES)

    # ---- DRAM I/O ----
    xt_d = nc.dram_tensor("xt", [SPC, 3, N], f32, kind="ExternalInput")
    wn_d, wv_d, s_d, b_d = [], [], [], []
    for li in range(4):
        C, Co = INC[li], HD[li]
        wn_d.append(nc.dram_tensor(f"wn{li}", [C, Co], f32, kind="ExternalInput"))
        wv_d.append(nc.dram_tensor(f"wv{li}", [C, Co], f32, kind="ExternalInput"))
        nch = (Co + 127) // 128
        s_d.append(nc.dram_tensor(f"s{li}", [min(Co, 128), nch], f32, kind="ExternalInput"))
        b_d.append(nc.dram_tensor(f"b{li}", [min(Co, 128), nch], f32, kind="ExternalInput"))
    # wf split per layer-chunk, transposed: [C_l, 1024]
    WFT_SPEC = [64, 64, 128, 128, 128]
    wft_d = [nc.dram_tensor(f"wft{i}", [ck, Z2], f32, kind="ExternalInput")
             for i, ck in enumerate(WFT_SPEC)]
    bf_d = nc.dram_tensor("bf2d", [128, 8], f32, kind="ExternalInput")
    eye_d = nc.dram_tensor("eye", [128, 128], f32, kind="ExternalInput")
    out_d = nc.dram_tensor("out", [SPC, 128, 8], f32, kind="ExternalOutput")
    u_hbm = [[nc.dram_tensor(f"u_{s}_{li}", [N, HD[li]], f32, kind="Internal")
              for li in range(4)] for s in range(SPC)]
    # per-tile gather index lists in the SWDGE 16-wrap layout [16, 320]
    ih_hbm = [[nc.dram_tensor(f"ih_{s}_{li}", [N // 128, 16, 320], u16,
                              kind="Internal") for li in range(4)]
              for s in range(SPC)]

    NT = N // 128  # 8 point-tiles

    with tile.TileContext(nc) as tc, ExitStack() as ctx:
        pool = ctx.enter_context(tc.tile_pool(name="main", bufs=1))
        pD = ctx.enter_context(tc.tile_pool(name="pD", bufs=4))
        pX = ctx.enter_context(tc.tile_pool(name="pX", bufs=2))
        pG = ctx.enter_context(tc.tile_pool(name="pG", bufs=2))
        pV = ctx.enter_context(tc.tile_pool(name="pV", bufs=2))
        pU = ctx.enter_context(tc.tile_pool(name="pU", bufs=2))
        pSm = ctx.enter_context(tc.tile_pool(name="pSm", bufs=3))
        pNs = ctx.enter_context(tc.tile_pool(name="pNs", bufs=1))
        pPd = ctx.enter_context(tc.tile_pool(name="pPd", bufs=1, space="PSUM"))
        pPm = ctx.enter_context(tc.tile_pool(name="pPm", bufs=2, space="PSUM"))
        pPb = ctx.enter_context(tc.tile_pool(name="pPb", bufs=1, space="PSUM"))

        # ---- persistent constants ----
        xt_sb = []
        for s in range(SPC):
            t = pool.tile([3, N], f32, tag=f"xt{s}")
            nc.sync.dma_start(t[:], xt_d[s])
            xt_sb.append(t)
        wn_sb, wv_sb, s_sb, b_sb = [], [], [], []
        for li in range(4):
            C, Co = INC[li], HD[li]
            t = pool.tile([C, Co], f32, tag=f"wn{li}")
            nc.sync.dma_start(t[:], wn_d[li][:])
            wn_sb.append(t)
            t = pool.tile([C, Co], f32, tag=f"wv{li}")
            nc.sync.dma_start(t[:], wv_d[li][:])
            wv_sb.append(t)
            nch = (Co + 127) // 128
            t = pool.tile([min(Co, 128), nch], f32, tag=f"s{li}")
            nc.sync.dma_start(t[:], s_d[li][:])
            s_sb.append(t)
            t = pool.tile([min(Co, 128), nch], f32, tag=f"b{li}")
            nc.sync.dma_start(t[:], b_d[li][:])
            b_sb.append(t)
        wft_sb = []
        for i, ck in enumerate(WFT_SPEC):
            t = pool.tile([ck, Z2], f32, tag=f"wft{i}")
            nc.sync.dma_start(t[:], wft_d[i][:])
            wft_sb.append(t)
        bf_sb = pool.tile([128, 8], f32, tag="bf")
        nc.sync.dma_start(bf_sb[:], bf_d[:])
        eye = pool.tile([128, 128], f32, tag="eye")
        nc.sync.dma_start(eye[:], eye_d[:])
        ones1 = pool.tile([1, 128], f32, tag="ones1")
        nc.vector.memset(ones1[:], 1.0)
        negones = pool.tile([128, 1], f32, tag="negones")
        nc.vector.memset(negones[:], -1.0)
        twos = pool.tile([128, 1], f32, tag="twos")
        nc.vector.memset(twos[:], 2.0)

        # layer outputs (transposed [C,1024]), persistent: per sample,
        # l1,l2 -> [64,1024]; l3 -> [128,1024]; l4 -> 2x[128,1024]
        cat_sb = [[pool.tile([64, N], f32, tag=f"o0_{s}", name=f"o0_{s}"),
                   pool.tile([64, N], f32, tag=f"o1_{s}", name=f"o1_{s}"),
                   pool.tile([128, N], f32, tag=f"o2_{s}", name=f"o2_{s}"),
                   pool.tile([128, N], f32, tag=f"o3a_{s}", name=f"o3a_{s}"),
                   pool.tile([128, N], f32, tag=f"o3b_{s}", name=f"o3b_{s}")]
                  for s in range(SPC)]

        def layer(s, li, XT):
            """XT: [C, 1024] sbuf AP for this layer's input (transposed)."""
            C, Co = INC[li], HD[li]
            nch = (Co + 127) // 128
            # ---- prep: Xsq, negsq, X2 ----
            xsq = pD.tile([C, N], f32, tag="d")
            nc.scalar.activation(xsq[:], XT, Act.Square)
            sq_ps = pPb.tile([1, N], f32, tag="big")
            for h in range(2):
                nc.tensor.matmul(sq_ps[:, 512 * h:512 * (h + 1)], negones[0:C, :],
                                 xsq[:, 512 * h:512 * (h + 1)], start=True, stop=True)
            nsq = pNs.tile([1, N], f32, tag="nsq")
            nc.scalar.activation(nsq[:], sq_ps[:], Act.Identity)
            x2 = pX.tile([C, N], f32, tag="x2")
            nc.scalar.activation(x2[:], XT, Act.Identity, scale=twos[0:C, :])

            # ---- U = X Wn^T  (point-major [128,Co] tiles) -> HBM ----
            uw_insts = []
            for t in range(NT):
                ups = pPb.tile([128, Co], f32, tag="big")
                nc.tensor.matmul(ups[:], XT[:, 128 * t:128 * (t + 1)], wn_sb[li][:],
                                 start=True, stop=True)
                usb = pU.tile([128, Co], f32, tag="u")
                nc.scalar.activation(usb[:], ups[:], Act.Identity)
                uw = nc.sync.dma_start(u_hbm[s][li][128 * t:128 * (t + 1), :], usb[:])
                uw_insts.append(uw)

            # ---- V' = s*V + b (transposed [Co,1024]) ----
            vp = []
            for a in range(nch):
                ca = min(128, Co - 128 * a)
                vps = pPb.tile([128, N], f32, tag="big")
                for h in range(2):
                    nc.tensor.matmul(vps[0:ca, 512 * h:512 * (h + 1)],
                                     wv_sb[li][:, 128 * a:128 * a + ca],
                                     XT[:, 512 * h:512 * (h + 1)],
                                     start=True, stop=True)
                vsb = pV.tile([ca, N], f32, tag="v")
                nc.scalar.activation(vsb[:], vps[0:ca, :], Act.Identity,
                                     scale=s_sb[li][0:ca, a:a + 1],
                                     bias=b_sb[li][0:ca, a:a + 1])
                vp.append(vsb)

            # ---- per point-tile: D, top-40, gather, neighbor max ----
            mps = [pPm.tile([128, N], f32, tag="m", name=f"m_{s}_{li}_{a}")
                   for a in range(nch)]
            for t in range(NT):
                dps = pPd.tile([128, N], f32, tag="dps")
                for h in range(2):
                    hs = slice(512 * h, 512 * (h + 1))
                    nc.tensor.matmul(dps[:, hs], x2[:, 128 * t:128 * (t + 1)],
                                     XT[:, hs], start=True, stop=False)
                    nc.tensor.matmul(dps[:, hs], ones1[:], nsq[:, hs],
                                     start=False, stop=True)
                dsb = pD.tile([128, N], f32, tag="d")
                nc.scalar.activation(dsb[:], dps[:], Act.Identity)
                idx = pSm.tile([128, 128], u16, tag="idx")
                nc.vector.memset(idx[:, KNN:], 0)
                for q in range(5):
                    mv = pSm.tile([128, 8], f32, tag="mv")
                    nc.vector.max(out=mv[:], in_=dsb[:])
                    nc.vector.max_index(out=idx[:, 8 * q:8 * q + 8], in_max=mv[:],
                                        in_values=dsb[:])
                    if q < 4:
                        nc.vector.match_replace(out=dsb[:], in_to_replace=mv[:],
                                                in_values=dsb[:], imm_value=NEG)
                # rewrap indices [128 pts, 40] -> SWDGE 16-wrap [16, 320] via
                # XBAR transpose + free-dim permute + HBM bounce
                idxT = pSm.tile([128, 128], u16, tag="idxT")
                nc.sync.dma_start_transpose(idxT[:], idx[:])
                idxP = pSm.tile([KNN, 128], u16, tag="idxP")
                nc.vector.tensor_copy(
                    idxP[:].rearrange("m (s c8) -> m s c8", c8=8),
                    idxT[0:KNN, :].rearrange("m (c8 s) -> m s c8", s=16))
                ih_v = ih_hbm[s][li][t].rearrange("s (m c8) -> m s c8", c8=8)
                ihw = nc.sync.dma_start(ih_v, idxP[:].rearrange(
                    "m (s c8) -> m s c8", c8=8))
                idxs_sb = pSm.tile([128, 320], i16, tag="idxsb")
                for g in range(8):
                    rd = nc.sync.dma_start(idxs_sb[16 * g:16 * (g + 1), :],
                                           ih_hbm[s][li][t].bitcast(i16))
                    _add_dep_helper(rd.ins, ihw.ins, sync=True, reason="ih RAW")
                gb = pG.tile([128, KNN * Co], f32, tag="gb")
                g3 = gb[:].rearrange("p (r c) -> p r c", r=KNN)
                for q in range(5):  # 1024 descriptors per SWDGE call (ring cap)
                    gi = nc.gpsimd.dma_gather(
                        out_ap=g3[:, 8 * q:8 * (q + 1), :], in_ap=u_hbm[s][li][:],
                        idxs_ap=idxs_sb[:, 64 * q:64 * (q + 1)],
                        num_idxs=1024, num_idxs_reg=1024, elem_size=Co)
                    for uw in uw_insts:
                        _add_dep_helper(gi.ins, uw.ins, sync=True, reason="u_hbm RAW")
                # max over 40 neighbor rows (DVE reduce over strided view)
                mrow = pU.tile([128, Co], f32, tag="mrow")
                nc.vector.tensor_reduce(
                    out=mrow[:], in_=gb[:].rearrange("p (r c) -> p c r", r=KNN),
                    axis=Ax.X, op=Alu.max)
                # transpose M rows [128, Co] -> M^T columns
                for a in range(nch):
                    ca = min(128, Co - 128 * a)
                    nc.tensor.transpose(mps[a][0:ca, 128 * t:128 * (t + 1)],
                                        mrow[:, 128 * a:128 * a + ca], eye[:])

            # ---- epilogue: out = lrelu(s*M^T + V') ----
            outs = []
            for a in range(nch):
                ca = min(128, Co - 128 * a)
                o = _otile(s, li, a)
                nc.vector.scalar_tensor_tensor(out=o[:], in0=mps[a][0:ca, :],
                                               scalar=s_sb[li][0:ca, a:a + 1],
                                               in1=vp[a][:], op0=Alu.mult, op1=Alu.add)
                nc.vector.scalar_tensor_tensor(out=o[:], in0=o[:], scalar=0.2,
                                               in1=o[:], op0=Alu.mult, op1=Alu.max)
                outs.append(o)
            return outs

        def _otile(s, li, a):
            return cat_sb[s][li + a if li < 3 else 3 + a]

        for li in range(4):
            for s in range(SPC):
                if li == 0:
                    layer(s, 0, xt_sb[s][:])
                elif li == 3:
                    layer(s, 3, cat_sb[s][2][:])
                else:
                    layer(s, li, cat_sb[s][li - 1][:])

        # ---- final: y = cat @ wf^T + bf ; out = max over points ----
        for s in range(SPC):
            ycat = pSm.tile([128, 8], f32, tag="ycat")
            rhs = [cat_sb[s][0], cat_sb[s][1], cat_sb[s][2], cat_sb[s][3], cat_sb[s][4]]
            for o in range(8):
                yps = pPb.tile([128, N], f32, tag="big")
                for h in range(2):
                    hs = slice(512 * h, 512 * (h + 1))
                    for k in range(5):
                        nc.tensor.matmul(yps[:, hs],
                                         wft_sb[k][:, 128 * o:128 * (o + 1)],
                                         rhs[k][:, hs], start=(k == 0),
                                         stop=(k == 4))
                nc.vector.tensor_reduce(out=ycat[:, o:o + 1], in_=yps[:],
                                        axis=Ax.X, op=Alu.max)
            nc.vector.tensor_tensor(out=ycat[:], in0=ycat[:], in1=bf_sb[:],
                                    op=Alu.add)
            nc.sync.dma_start(out_d[s], ycat[:])

    nc.compile()
    return nc


def _host_inputs(inputs):
    """Build the per-core input maps from the full problem inputs."""
    x = np.asarray(inputs["x"], dtype=np.float32)
    maps = []
    base = {}
    for li in range(4):
        C, Co = INC[li], HD[li]
        w = np.asarray(inputs[f"w{li + 1}"], dtype=np.float32)
        g = np.asarray(inputs[f"g{li + 1}"], dtype=np.float32)
        b = np.asarray(inputs[f"b{li + 1}"], dtype=np.float32)
        wn = w[:, :C]
        wv = w[:, C:] - wn
        base[f"wn{li}"] = np.ascontiguousarray(wn.T)
        base[f"wv{li}"] = np.ascontiguousarray(wv.T)
        nch = (Co + 127) // 128
        base[f"s{li}"] = np.ascontiguousarray(
            (g * BN_INV).astype(np.float32).reshape(nch, -1).T)
        base[f"b{li}"] = np.ascontiguousarray(b.reshape(nch, -1).T)
    wf = np.asarray(inputs["wf"], dtype=np.float32)
    offs = np.cumsum([0] + HD)
    pieces = [wf[:, offs[i]:offs[i + 1]] for i in range(4)]
    wfts = [pieces[0], pieces[1], pieces[2], pieces[3][:, :128], pieces[3][:, 128:]]
    for i, p in enumerate(wfts):
        base[f"wft{i}"] = np.ascontiguousarray(p.T)
    bf = np.asarray(inputs["bf"], dtype=np.float32)
    base["bf2d"] = np.ascontiguousarray(bf.reshape(8, 128).T)
    base["eye"] = np.eye(128, dtype=np.float32)
    for c in range(NCORES):
        m = dict(base)
        xs = x[SPC * c:SPC * (c + 1)]  # [SPC, N, 3]
        m["xt"] = np.ascontiguousarray(np.transpose(xs, (0, 2, 1)))
        maps.append(m)
    return maps


def kernel(**inputs):
    from concourse.bass_utils import run_bass_kernel_spmd

    if "nc" not in _CACHE:
        _CACHE["nc"] = _build_program()
    nc = _CACHE["nc"]
    in_maps = _host_inputs(inputs)
    res = run_bass_kernel_spmd(nc, in_maps, list(range(NCORES)))
    out = np.empty((B, Z2), dtype=np.float32)
    for c in range(NCORES):
        y = res.results[c]["out"]  # [SPC, 128, 8]
        for s in range(SPC):
            out[SPC * c + s] = y[s].T.reshape(Z2)
    return out
